# revision 16
# baseline (speedup 1.0000x reference)
"""MoE ExpertPool kernel for 8 Trainium2 NeuronCores (expert-parallel).

Host side: one expert per core.  Tokens routed to expert e (via either
top-k slot) are gathered and padded to a common capacity C (multiple of
128, split into two equal token chunks <= 512).  All device tensors are
pre-arranged on the host so every DMA is contiguous per partition and
every matmul uses natural [K, M] layouts:

  device (per core):  H = silu(Wg^T @ xT) * (Wu^T @ xT)      [d_expert, C]
                      yT = Wd^T @ H                          [d_model, C]

Activations stay transposed ([feature, token]) the whole way, so the
tokens live on the matmul free dim and weights are the stationary lhsT.
The per-token routing weight and the scatter-add back to (B,S,D) happen
on the host (they are linear post-ops of yT).

Everything runs in bfloat16 (fp32 PSUM accumulation): same PE rate as
f32r but weight DMA traffic halves and LDWEIGHTS gets the fast-weight-
load (FWL) path, so the weight loads fully hide under the matmul
stream.  Measured absmax relative error ~4e-3 (tolerance 2e-2).
"""

import numpy as np

D_MODEL = 768
D_EXPERT = 3072
N_EXPERTS = 8
TOP_K = 2
P = 128
KD = D_MODEL // P      # 6   d_model chunks of 128
MD = D_EXPERT // P     # 24  d_expert chunks of 128
WG_W = 256             # gate/up stationary-weight tile width
N_WG = D_EXPERT // WG_W
WARM = 56              # thin (N=64) PE pre-warm matmuls (~3us @ cold clock)

_CACHE = {}
LAST_RESULTS = None


def _ensure_axon_hooks():
    """Provide antenv.axon_hooks if the image lacks it, so the trace=True
    path of run_bass_kernel_spmd works (and BASS_TRACE=1 can't crash us)."""
    import sys
    import types

    try:
        import antenv.axon_hooks  # noqa: F401

        return
    except ImportError:
        pass
    try:
        import antenv
    except ImportError:
        return
    mod = types.ModuleType("antenv.axon_hooks")
    mod._hook = None
    mod.set_axon_ntff_profile_hook = lambda h: setattr(mod, "_hook", h)
    mod.get_axon_ntff_profile_hook = lambda: mod._hook
    sys.modules["antenv.axon_hooks"] = mod
    antenv.axon_hooks = mod
    try:
        from trn_agent_boot.trn_boot import _ntff_profile_via_ctypes

        hook = _ntff_profile_via_ctypes("/opt/axon/libaxon_pjrt.so")
        if hook is not None:
            mod._hook = hook
    except Exception:
        pass


def _build(C):
    import concourse.mybir as mybir
    import concourse.tile as tile
    from concourse import bacc

    f32 = mybir.dt.float32
    f16 = mybir.dt.float16
    bf16 = mybir.dt.bfloat16
    Act = mybir.ActivationFunctionType

    TCH = C // 2           # token chunk; C <= 1024 so TCH <= 512 (PSUM bank)
    NB = 2
    assert C % 2 == 0 and TCH <= 512

    nc = bacc.Bacc("TRN2", dynamic_dma_scratch_size=512, num_swdge_queues=1)
    xt = nc.dram_tensor("xt", [P, KD, C], bf16, kind="ExternalInput")
    wg = nc.dram_tensor("wg", [P, N_WG, KD, WG_W], bf16, kind="ExternalInput")
    wu = nc.dram_tensor("wu", [P, N_WG, KD, WG_W], bf16, kind="ExternalInput")
    wd = nc.dram_tensor("wd", [P, KD, MD, P], bf16, kind="ExternalInput")
    yt = nc.dram_tensor("yt", [P, KD, C], f16, kind="ExternalOutput")

    with tile.TileContext(nc) as tc:
        with (
            tc.tile_pool(name="singles", bufs=1) as singles,
            tc.tile_pool(name="wpool", bufs=2) as wpool,
            tc.tile_pool(name="tmp", bufs=3) as tmp,
            tc.tile_pool(name="psum", bufs=2, space="PSUM") as psum,
        ):
            xt_sb = singles.tile([P, KD, C], bf16)
            H_sb = singles.tile([P, MD, C], bf16)

            # Prologue: xt k-chunks alternate between the two HWDGE queues
            # (SP + ACT), interleaved with the mo=0 weight chunks, so the
            # first matmul starts right after the framework preamble and the
            # m=0 k-loop is fed at DMA pace without enqueue-rate overhead.
            wg_t0 = wpool.tile([P, KD, WG_W], bf16, tag="wg", bufs=2, name="wg_t0")
            wu_t0 = wpool.tile([P, KD, WG_W], bf16, tag="wu", bufs=2, name="wu_t0")
            # Whole k-triples alternate queues (k=0 entirely on SP, first)
            # so the first matmul is gated by a single queue's spin-up —
            # the two HWDGE queues' DGE pipelines start up to ~1.2us apart.
            qrr = [nc.sync, nc.scalar]
            for k in range(KD):
                q = qrr[k % 2]
                q.dma_start(out=wg_t0[:, k], in_=wg[:, 0, k])
                for b in range(NB):
                    q.dma_start(
                        out=xt_sb[:, k, b * TCH : (b + 1) * TCH],
                        in_=xt[:, k, b * TCH : (b + 1) * TCH],
                    )
                q.dma_start(out=wu_t0[:, k], in_=wu[:, 0, k])

            # PE pre-warm: dummy matmuls on a zeroed tile while the first
            # DMAs are in flight, so HAM is at K=8/8 when real matmuls start.
            # memset on GpSimd: it is idle in the framework preamble, so the
            # first warm matmul can issue the moment the PE queue opens.
            warm_sb = singles.tile([P, 64], bf16, name="warm_sb")
            nc.gpsimd.memset(warm_sb[:], 0.0)
            warm_ps = psum.tile([P, 64], f32, tag="ups", bufs=4, name="warm_ps")
            for _ in range(WARM):
                nc.tensor.matmul(
                    warm_ps[:64], warm_sb[:, :64], warm_sb[:], start=True, stop=True
                )

            # First two m-groups interleave gate and up per k, paced by the
            # chunk arrivals above; their PSUM groups stay open through the
            # whole xt load so the PE does real work during the DMA window.
            part = []
            for mj in range(2):
                m = mj
                ms = slice(mj * P, (mj + 1) * P)
                g_ps = [
                    psum.tile([P, TCH], f32, tag="gps", bufs=4, name=f"g_{m}_{b}")
                    for b in range(NB)
                ]
                u_ps = [
                    psum.tile([P, TCH], f32, tag="ups", bufs=4, name=f"u_{m}_{b}")
                    for b in range(NB)
                ]
                part.append((m, ms, g_ps, u_ps))
            for k in range(KD):
                st, sp = k == 0, k == KD - 1
                for b in range(NB):
                    for m, ms, g_ps, u_ps in part:
                        nc.tensor.matmul(
                            g_ps[b], wg_t0[:, k, ms],
                            xt_sb[:, k, b * TCH : (b + 1) * TCH],
                            start=st, stop=sp,
                        )
                for b in range(NB):
                    for m, ms, g_ps, u_ps in part:
                        nc.tensor.matmul(
                            u_ps[b], wu_t0[:, k, ms],
                            xt_sb[:, k, b * TCH : (b + 1) * TCH],
                            start=st, stop=sp,
                        )
            for m, ms, g_ps, u_ps in part:
                sils = []
                for b in range(NB):
                    sil = tmp.tile([P, TCH], bf16, tag="sil", bufs=2,
                                   name=f"sil_p{m}_{b}")
                    nc.scalar.activation(out=sil[:], in_=g_ps[b], func=Act.Silu)
                    sils.append(sil)
                for b in range(NB):
                    nc.vector.tensor_mul(
                        H_sb[:, m, b * TCH : (b + 1) * TCH], sils[b], u_ps[b]
                    )

            # gate/up projections + silu*mul -> H   (d_expert = m*128 + p).
            for mo in range(N_WG):
                if mo == 0:
                    wg_t, wu_t = wg_t0, wu_t0
                else:
                    wg_t = wpool.tile([P, KD, WG_W], bf16, tag="wg", bufs=2)
                    nc.sync.dma_start(out=wg_t[:], in_=wg[:, mo])
                    wu_t = wpool.tile([P, KD, WG_W], bf16, tag="wu", bufs=2)
                    nc.scalar.dma_start(out=wu_t[:], in_=wu[:, mo])
                for mj in range(WG_W // P):
                    m = mo * (WG_W // P) + mj
                    if m < 2:
                        continue
                    ms = slice(mj * P, (mj + 1) * P)
                    g_ps = [
                        psum.tile([P, TCH], f32, tag="gps", bufs=4,
                                  name=f"g_{m}_{b}")
                        for b in range(NB)
                    ]
                    u_ps = [
                        psum.tile([P, TCH], f32, tag="ups", bufs=4,
                                  name=f"u_{m}_{b}")
                        for b in range(NB)
                    ]
                    for k in range(KD):
                        st, sp = k == 0, k == KD - 1
                        for b in range(NB):
                            nc.tensor.matmul(
                                g_ps[b],
                                wg_t[:, k, ms],
                                xt_sb[:, k, b * TCH : (b + 1) * TCH],
                                start=st, stop=sp,
                            )
                    sils = []
                    for b in range(NB):
                        sil = tmp.tile([P, TCH], bf16, tag="sil", bufs=2,
                                       name=f"sil_{m}_{b}")
                        nc.scalar.activation(out=sil[:], in_=g_ps[b], func=Act.Silu)
                        sils.append(sil)
                    for k in range(KD):
                        st, sp = k == 0, k == KD - 1
                        for b in range(NB):
                            nc.tensor.matmul(
                                u_ps[b],
                                wu_t[:, k, ms],
                                xt_sb[:, k, b * TCH : (b + 1) * TCH],
                                start=st, stop=sp,
                            )
                    for b in range(NB):
                        nc.vector.tensor_mul(
                            H_sb[:, m, b * TCH : (b + 1) * TCH], sils[b], u_ps[b]
                        )

            # down projection   (d_model = n*128 + p); reuses the gps PSUM
            # tag so the kernel stays within 8 banks.  The last n-group runs
            # as two wide chunks followed by a 128-token sliver: the wide
            # chunks' output DMAs (the bulk of the final bytes) drain while
            # the sliver's matmuls run, so the post-matmul tail is only a
            # 32KB write.
            MDH = MD // 2
            SL = 128 if C > 256 else 0
            for n in range(KD):
                last = n == KD - 1
                wd_h = []
                for h in range(2):
                    wd_t = wpool.tile(
                        [P, MDH, P], bf16, tag="wd", bufs=2, name=f"wd_{n}_{h}"
                    )
                    eng = nc.sync if h == 0 else nc.scalar
                    eng.dma_start(
                        out=wd_t[:], in_=wd[:, n, h * MDH : (h + 1) * MDH]
                    )
                    wd_h.append(wd_t)
                if last and SL:
                    # serialize the chunks: A's copy+DMA drains during B's
                    # matmuls, B's during the sliver's, so only the sliver's
                    # small write trails the final matmul
                    spans = [(0, TCH), (TCH, C - SL), (C - SL, C)]
                    for b, (s, e) in enumerate(spans):
                        y_ps = psum.tile([P, e - s], f32, tag="gps", bufs=4,
                                         name=f"y_{n}_{b}")
                        for k in range(MD):
                            st, sp = k == 0, k == MD - 1
                            lhs = wd_h[k // MDH][:, k % MDH, :]
                            nc.tensor.matmul(
                                y_ps, lhs, H_sb[:, k, s:e], start=st, stop=sp,
                            )
                        y_sb = tmp.tile([P, e - s], f16, tag="ysbq", bufs=3,
                                        name=f"ysb_{n}_{b}")
                        (nc.vector.tensor_copy if b % 2 == 0 else nc.scalar.copy)(
                            out=y_sb[:], in_=y_ps
                        )
                        (nc.sync if b % 2 == 0 else nc.scalar).dma_start(
                            out=yt[:, n, s:e], in_=y_sb[:]
                        )
                    continue
                spans = [(0, TCH), (TCH, C)]
                y_ps = [
                    psum.tile([P, e - s], f32, tag="gps", bufs=4,
                              name=f"y_{n}_{b}")
                    for b, (s, e) in enumerate(spans)
                ]
                for k in range(MD):
                    st, sp = k == 0, k == MD - 1
                    lhs = wd_h[k // MDH][:, k % MDH, :]
                    for b, (s, e) in enumerate(spans):
                        nc.tensor.matmul(
                            y_ps[b], lhs, H_sb[:, k, s:e], start=st, stop=sp,
                        )
                for b, (s, e) in enumerate(spans):
                    y_sb = tmp.tile([P, e - s], f16, tag="ysb", bufs=2,
                                    name=f"ysb_{n}_{b}")
                    nc.any.tensor_copy(out=y_sb[:], in_=y_ps[b])
                    (nc.sync if b % 2 == 0 else nc.scalar).dma_start(
                        out=yt[:, n, s:e], in_=y_sb[:]
                    )
    nc.finalize()
    return nc


def kernel(**inputs):
    global LAST_RESULTS
    import ml_dtypes

    bf = ml_dtypes.bfloat16
    x = np.ascontiguousarray(np.asarray(inputs["x"], dtype=np.float32))
    rw = np.asarray(inputs["routing_weights"], dtype=np.float32)
    ei = np.asarray(inputs["expert_indices"])
    wg = np.asarray(inputs["w_gate"], dtype=np.float32)
    wu = np.asarray(inputs["w_up"], dtype=np.float32)
    wd = np.asarray(inputs["w_down"], dtype=np.float32)

    B, S, D = x.shape
    T = B * S
    xf = x.reshape(T, D)
    eif = ei.reshape(T, TOP_K).astype(np.int64)
    rwf = rw.reshape(T, TOP_K)

    # per-token weight for each expert (sum over top-k slots assigned to e)
    tokw = np.zeros((T, N_EXPERTS), np.float32)
    np.add.at(tokw, (np.arange(T)[:, None], eif), rwf)

    idxs = [np.nonzero((eif == e).any(axis=1))[0] for e in range(N_EXPERTS)]
    # Capacity: smallest multiple of 64 in [512, 1024] that spills at most
    # ~2% of routed tokens to the (exact) host path — streamed columns are
    # the dominant device cost, so C directly scales kernel time.  Capped at
    # 1024 so xt+H stay within SBUF and C/2 fits one PSUM bank.
    routed = sum(len(i) for i in idxs)
    budget = max(P, routed * 2 // 100)
    C = 1024
    for cand in range(512, 1025, 64):
        if sum(max(0, len(i) - cand) for i in idxs) <= budget:
            C = cand
            break

    _ensure_axon_hooks()
    from concourse.bass_utils import run_bass_kernel_spmd

    nc = _CACHE.get(C)
    if nc is None:
        nc = _CACHE[C] = _build(C)

    wgb = wg.astype(bf)
    wub = wu.astype(bf)
    wdb = wd.astype(bf)
    in_maps = []
    for e in range(N_EXPERTS):
        idx = idxs[e][:C]
        xe = np.zeros((C, D), bf)
        xe[: len(idx)] = xf[idx].astype(bf)
        in_maps.append(
            {
                "xt": np.ascontiguousarray(xe.T.reshape(KD, P, C).transpose(1, 0, 2)),
                "wg": np.ascontiguousarray(
                    wgb[e].reshape(KD, P, N_WG, WG_W).transpose(1, 2, 0, 3)
                ),
                "wu": np.ascontiguousarray(
                    wub[e].reshape(KD, P, N_WG, WG_W).transpose(1, 2, 0, 3)
                ),
                "wd": np.ascontiguousarray(
                    wdb[e].reshape(MD, P, KD, P).transpose(1, 2, 0, 3)
                ),
            }
        )

    try:
        res = run_bass_kernel_spmd(nc, in_maps, core_ids=list(range(N_EXPERTS)))
    except Exception:
        # transient NRT/device hiccups (e.g. NRT_EXEC_UNIT_UNRECOVERABLE)
        # usually clear on a retry
        res = run_bass_kernel_spmd(nc, in_maps, core_ids=list(range(N_EXPERTS)))
    LAST_RESULTS = res

    out = np.zeros((T, D), np.float32)
    for e in range(N_EXPERTS):
        idx = idxs[e][:C]
        ye = (
            np.asarray(res.results[e]["yt"], dtype=np.float32)
            .transpose(1, 0, 2)
            .reshape(D, C)
            .T
        )
        out[idx] += ye[: len(idx)] * tokw[idx, e][:, None]
        spill = idxs[e][C:]
        if len(spill):
            xs = xf[spill]
            h = xs @ wg[e]
            h = (h / (1.0 + np.exp(-h))) * (xs @ wu[e])
            out[spill] += (h @ wd[e]) * tokw[spill, e][:, None]
    return out.reshape(B, S, D)
